# revision 1
# baseline (speedup 1.0000x reference)
"""GCN encoder (2-layer GCNConv) as a Bass/Tile kernel on 8 Trainium2 NeuronCores.

Strategy (matches the sharding hint):
  - Nodes row-partitioned across 8 cores (6250 rows each); weights replicated.
  - Symmetric normalization factorized: z = D^-1/2 (A+I) D^-1/2 (x W) + b
    =>  u = dinv * (x W);  agg[d] = u[d] + sum_{e:dst=d} u[src_e];
        z = dinv * agg + b
    so no per-edge norm gather is needed.
  - Per layer: local matmul -> row scale -> AllGather(u) -> per-core gather of
    source rows (dma_gather) -> segment-sum via tensor-engine matmuls with
    compile-time-structured 0/1 selection matrices generated on DVE
    (is_equal against an iota) -> scale/bias/relu -> output rows.
  - Edges are bucketed host-side by (dst window of 128, src half) and padded to
    128-slot tiles; padded slots gather row 0 and have an all-zero selection
    column, so they contribute nothing.  int16 gather indices require the
    src-half split (indices < 32768).
"""

import math
import os
import sys

import numpy as np

sys.path.insert(0, "/opt/trn_rl_repo")

import ml_dtypes

BF16 = ml_dtypes.bfloat16


class Cfg:
    def __init__(self, N, E, IN=512, HID=256, OUT=128, P=8, half=None):
        self.N, self.E, self.IN, self.HID, self.OUT, self.P = N, E, IN, HID, OUT, P
        self.NC = N // P                      # nodes per core
        self.WS = 128                         # dst window size
        self.NW = math.ceil(self.NC / self.WS)  # windows per core
        # src-half split point (int16 gather indices must stay < 32768)
        if half is None:
            half = N if N <= 32767 else (N + 1) // 2
        self.HALF = half
        assert self.HALF <= 32767 and N - self.HALF <= 32767


FULL = Cfg(N=50000, E=800000)


def _prepare(cfg, x, edge_index, W1, b1, W2, b2):
    """Host-side graph preprocessing -> per-core input maps + program params."""
    N, P, NC, WS, NW, HALF = cfg.N, cfg.P, cfg.NC, cfg.WS, cfg.NW, cfg.HALF
    src = np.asarray(edge_index[0], dtype=np.int64)
    dst = np.asarray(edge_index[1], dtype=np.int64)

    deg = np.bincount(dst, minlength=N).astype(np.float64) + 1.0  # + self loop
    dinv = (1.0 / np.sqrt(deg)).astype(np.float32)

    # group id: ((core, window), src-half) ; groups contiguous after sort
    win_id = (dst // NC) * NW + (dst % NC) // WS
    half = (src >= HALF).astype(np.int64)
    comp = win_id * 2 + half
    order = np.argsort(comp, kind="stable")
    s_s, d_s, c_s = src[order], dst[order], comp[order]
    counts = np.bincount(c_s, minlength=P * NW * 2).reshape(P, NW, 2)

    # shared tile counts per (window, half): max over cores
    T = np.ceil(counts.max(axis=0) / 128).astype(np.int64)  # [NW, 2]
    tiles_total = int(T.sum())
    slots_total = tiles_total * 128

    starts = np.zeros(P * NW * 2 + 1, dtype=np.int64)
    np.cumsum(counts.reshape(-1), out=starts[1:])

    dinv_pad = np.concatenate(
        [dinv, np.ones(NW * WS * P - N, dtype=np.float32)])

    in_maps = []
    for c in range(P):
        idx_arr = np.zeros(slots_total, dtype=np.int16)
        aco_arr = np.full(slots_total, -1, dtype=np.float32)  # cast to bf16 below
        off = 0
        for w in range(NW):
            for h in range(2):
                g = (c * NW + w) * 2 + h
                n = counts[c, w, h]
                sl = slice(starts[g], starts[g] + n)
                idx_arr[off:off + n] = (s_s[sl] - h * HALF).astype(np.int16)
                aco_arr[off:off + n] = (d_s[sl] - c * NC - w * WS).astype(np.float32)
                off += 128 * int(T[w, h])
        assert off == slots_total

        dloc = np.concatenate(
            [dinv[c * NC:(c + 1) * NC],
             np.ones(NW * WS - NC, dtype=np.float32)])

        m = {
            "xT": np.ascontiguousarray(
                np.asarray(x[c * NC:(c + 1) * NC], np.float32).astype(BF16).T),
            "w1": np.ascontiguousarray(
                np.asarray(W1, np.float32).astype(BF16)
                .reshape(cfg.IN // 128, 128, cfg.HID).transpose(1, 0, 2)),
            "w2": np.ascontiguousarray(
                np.asarray(W2, np.float32).astype(BF16)
                .reshape(cfg.HID // 128, 128, cfg.OUT).transpose(1, 0, 2)),
            "dinvc": np.ascontiguousarray(dloc.reshape(NW, WS).T),
            "idx": np.ascontiguousarray(np.tile(idx_arr.reshape(-1, 16).T, (8, 1))),
            "acol": np.ascontiguousarray(aco_arr.reshape(-1, 128).T.astype(BF16)),
            "ident": np.eye(128, dtype=BF16),
        }
        b1nz = bool(np.any(np.asarray(b1)))
        b2nz = bool(np.any(np.asarray(b2)))
        if b1nz:
            m["b1bc"] = np.ascontiguousarray(
                np.broadcast_to(np.asarray(b1, np.float32), (128, cfg.HID)))
        if b2nz:
            m["b2bc"] = np.ascontiguousarray(
                np.broadcast_to(np.asarray(b2, np.float32), (128, cfg.OUT)))
        in_maps.append(m)

    return in_maps, T, b1nz, b2nz


def build_program(cfg, T, b1nz, b2nz):
    import concourse.bass as bass
    import concourse.bacc as bacc
    import concourse.mybir as mybir
    from concourse import tile

    N, P, NC, WS, NW = cfg.N, cfg.P, cfg.NC, cfg.WS, cfg.NW
    IN, HID, OUT = cfg.IN, cfg.HID, cfg.OUT
    NCI, NCH = IN // 128, HID // 128
    tiles_total = int(T.sum())
    slots_total = tiles_total * 128
    f32, bf16, i16 = mybir.dt.float32, mybir.dt.bfloat16, mybir.dt.int16
    AF = mybir.ActivationFunctionType

    nc = bacc.Bacc("TRN2", target_bir_lowering=False, debug=False,
                   num_devices=cfg.P)
    xT_p = nc.dram_tensor("xT", [IN, NC], bf16, kind="ExternalInput")
    w1_p = nc.dram_tensor("w1", [128, NCI, HID], bf16, kind="ExternalInput")
    w2_p = nc.dram_tensor("w2", [128, NCH, OUT], bf16, kind="ExternalInput")
    dinv_p = nc.dram_tensor("dinvc", [WS, NW], f32, kind="ExternalInput")
    idx_p = nc.dram_tensor("idx", [128, slots_total // 16], i16, kind="ExternalInput")
    acol_p = nc.dram_tensor("acol", [128, tiles_total], bf16, kind="ExternalInput")
    id_p = nc.dram_tensor("ident", [128, 128], bf16, kind="ExternalInput")
    b1_p = (nc.dram_tensor("b1bc", [128, HID], f32, kind="ExternalInput")
            if b1nz else None)
    b2_p = (nc.dram_tensor("b2bc", [128, OUT], f32, kind="ExternalInput")
            if b2nz else None)
    out_p = nc.dram_tensor("out", [NC, OUT], f32, kind="ExternalOutput")

    u1d = nc.dram_tensor("u1d", [NC, HID], bf16)
    u2d = nc.dram_tensor("u2d", [NC, OUT], bf16)
    U1 = nc.dram_tensor("U1", [N, HID], bf16)
    U2 = nc.dram_tensor("U2", [N, OUT], bf16)
    rg = [list(range(P))]

    with tile.TileContext(nc) as tc:
        with (
            tc.tile_pool(name="res", bufs=1) as res,
            tc.tile_pool(name="work", bufs=4) as work,
            tc.tile_pool(name="gath", bufs=4) as gath,
            tc.tile_pool(name="psum", bufs=2, space="PSUM") as psum,
        ):
            # ---- resident loads ----
            xTs = res.tile([128, NCI, NC], bf16)
            for ci in range(NCI):
                nc.sync.dma_start(xTs[:, ci, :], xT_p[ci * 128:(ci + 1) * 128, :])
            w1s = res.tile([128, NCI, HID], bf16)
            nc.sync.dma_start(w1s[:], w1_p[:])
            w2s = res.tile([128, NCH, OUT], bf16)
            nc.sync.dma_start(w2s[:], w2_p[:])
            dinvs = res.tile([WS, NW], f32)
            nc.sync.dma_start(dinvs[:], dinv_p[:])
            idxs = res.tile([128, slots_total // 16], i16)
            nc.sync.dma_start(idxs[:], idx_p[:])
            acols = res.tile([128, tiles_total], bf16)
            nc.sync.dma_start(acols[:], acol_p[:])
            ident = res.tile([128, 128], bf16)
            nc.sync.dma_start(ident[:], id_p[:])
            iot = res.tile([128, 128], bf16)
            nc.gpsimd.iota(iot[:], pattern=[[1, 128]], base=0,
                           channel_multiplier=0,
                           allow_small_or_imprecise_dtypes=True)
            b1bc = None
            if b1nz:
                b1bc = res.tile([128, HID], f32)
                nc.sync.dma_start(b1bc[:], b1_p[:])
            b2bc = None
            if b2nz:
                b2bc = res.tile([128, OUT], f32)
                nc.sync.dma_start(b2bc[:], b2_p[:])

            u1res = res.tile([128, NW, HID], bf16)
            u2res = res.tile([128, NW, OUT], bf16)
            h1T = res.tile([128, NCH, NC], bf16)
            if NC % WS:
                # tail rows of the last window feed the self-loop matmul as
                # rhs; zero them so uninitialized SBUF can't inject NaNs
                nc.gpsimd.memset(u1res[:, NW - 1, :], 0.0)
                nc.gpsimd.memset(u2res[:, NW - 1, :], 0.0)

            def nsz(j):
                return min(128, NC - j * WS)

            MAXP = int(os.environ.get("GCN_MAX_PHASE", "9"))

            def emit_debug_out(src_bf16_ap, w, n):
                # convert [n, OUT] bf16 -> f32, dump into out rows of window w
                dt = work.tile([128, OUT], f32, tag="dbg")
                nc.scalar.activation(dt[:n, :], src_bf16_ap, AF.Copy)
                nc.sync.dma_start(out_p[w * WS:w * WS + n, :], dt[:n, :])

            # ---- phase A: t1 = x @ W1 ; u1 = dinv * t1 ----
            for j in range(NW):
                n = nsz(j)
                jsl = slice(j * WS, j * WS + n)
                pt = psum.tile([128, HID], f32, tag="mm")
                for ci in range(NCI):
                    nc.tensor.matmul(pt[:n, :], xTs[:, ci, jsl],
                                     w1s[:, ci, :], start=(ci == 0),
                                     stop=(ci == NCI - 1))
                nc.scalar.activation(u1res[:n, j, :], pt[:n, :], AF.Copy,
                                     scale=dinvs[:n, j:j + 1])
                nc.sync.dma_start(u1d[jsl, :], u1res[:n, j, :])
                if MAXP == 1:
                    emit_debug_out(u1res[:n, j, :OUT], j, n)
            if MAXP <= 1:
                return nc

            # ---- AllGather u1 ----
            nc.gpsimd.collective_compute(
                "AllGather", mybir.AluOpType.bypass, replica_groups=rg,
                ins=[u1d[:]], outs=[U1[:]])
            if MAXP == 2:
                for j in range(NW):
                    n = nsz(j)
                    gt = work.tile([128, OUT], bf16, tag="dbg_g")
                    nc.sync.dma_start(gt[:n, :], U1[j * WS:j * WS + n, :OUT])
                    emit_debug_out(gt[:n, :], j, n)
                return nc

            # ---- generic aggregation layer ----
            def agg_layer(U, F, ures, bbc, relu, emit_out):
                tile_idx = 0
                slot_off = 0
                for w in range(NW):
                    n = nsz(w)
                    pa = psum.tile([128, F], f32, tag="agg")
                    # self-loop term: ident.T @ u[w]
                    nc.tensor.matmul(pa[:n, :], ident[:, :n], ures[:, w, :],
                                     start=True, stop=False)
                    nmm = int(T[w, 0] + T[w, 1])
                    done = 0
                    for h in range(2):
                        t_wh = int(T[w, h])
                        if t_wh == 0:
                            continue
                        g = gath.tile([128, t_wh, F], bf16, tag="g")
                        base = 0 if h == 0 else cfg.HALF
                        nc.gpsimd.dma_gather(
                            g[:], U[base:base + min(cfg.HALF, N - base), :],
                            idxs[:, slot_off // 16:
                                 (slot_off + 128 * t_wh) // 16],
                            num_idxs=128 * t_wh, num_idxs_reg=128 * t_wh,
                            elem_size=F, single_packet=False)
                        slot_off += 128 * t_wh
                        for t in range(t_wh):
                            S = work.tile([128, 128], bf16, tag="S")
                            nc.vector.tensor_tensor(
                                S[:], iot[:],
                                acols[:, tile_idx:tile_idx + 1]
                                .broadcast_to((128, 128)),
                                op=mybir.AluOpType.is_equal)
                            tile_idx += 1
                            done += 1
                            nc.tensor.matmul(pa[:n, :], S[:, :n], g[:, t, :],
                                             start=False, stop=(done == nmm))
                    # z = dinv * agg (+ b) ; relu
                    if bbc is None:
                        zf = AF.Relu if relu else AF.Copy
                        zt = work.tile([128, F], f32 if emit_out else bf16,
                                       tag="zt%d" % F)
                        nc.scalar.activation(zt[:n, :], pa[:n, :], zf,
                                             scale=dinvs[:n, w:w + 1])
                    else:
                        v = work.tile([128, F], f32, tag="v%d" % F)
                        nc.scalar.activation(v[:n, :], pa[:n, :], AF.Copy,
                                             scale=dinvs[:n, w:w + 1])
                        zt = work.tile([128, F], f32 if emit_out else bf16,
                                       tag="zt%d" % F)
                        if relu:
                            vb = work.tile([128, F], f32, tag="vb%d" % F)
                            nc.vector.tensor_tensor(
                                vb[:n, :], v[:n, :], bbc[:n, :],
                                op=mybir.AluOpType.add)
                            nc.scalar.activation(zt[:n, :], vb[:n, :], AF.Relu)
                        else:
                            nc.vector.tensor_tensor(
                                zt[:n, :], v[:n, :], bbc[:n, :],
                                op=mybir.AluOpType.add)
                    yield w, n, zt

            # ---- phase C: layer-1 aggregation -> h1 -> h1T ----
            for w, n, zt in agg_layer(U1, HID, u1res, b1bc, True, False):
                wsl = slice(w * WS, w * WS + n)
                for ch in range(NCH):
                    ptr = psum.tile([128, 128], bf16, tag="tr")
                    nc.tensor.transpose(ptr[:, :n],
                                        zt[:n, ch * 128:(ch + 1) * 128],
                                        ident[:n, :n])
                    nc.scalar.activation(h1T[:, ch, wsl], ptr[:, :n], AF.Copy)
                if MAXP == 3:
                    emit_debug_out(zt[:n, :OUT], w, n)
            if MAXP <= 3:
                return nc

            # ---- phase D: t2 = h1 @ W2 ; u2 ----
            for j in range(NW):
                n = nsz(j)
                jsl = slice(j * WS, j * WS + n)
                pt = psum.tile([128, OUT], f32, tag="mm")
                for ch in range(NCH):
                    nc.tensor.matmul(pt[:n, :], h1T[:, ch, jsl],
                                     w2s[:, ch, :], start=(ch == 0),
                                     stop=(ch == NCH - 1))
                nc.scalar.activation(u2res[:n, j, :], pt[:n, :], AF.Copy,
                                     scale=dinvs[:n, j:j + 1])
                nc.sync.dma_start(u2d[jsl, :], u2res[:n, j, :])
                if MAXP == 4:
                    emit_debug_out(u2res[:n, j, :], j, n)
            if MAXP <= 4:
                return nc

            # ---- AllGather u2 ----
            nc.gpsimd.collective_compute(
                "AllGather", mybir.AluOpType.bypass, replica_groups=rg,
                ins=[u2d[:]], outs=[U2[:]])

            # ---- phase F: layer-2 aggregation -> out ----
            for w, n, zt in agg_layer(U2, OUT, u2res, b2bc, False, True):
                wsl = slice(w * WS, w * WS + n)
                nc.sync.dma_start(out_p[wsl, :], zt[:n, :])

    return nc


def run(cfg, inputs, sim=False, trace=False):
    from concourse.bass_utils import run_bass_kernel_spmd

    in_maps, T, b1nz, b2nz = _prepare(
        cfg, inputs["x"], inputs["edge_index"], inputs["W1"], inputs["b1"],
        inputs["W2"], inputs["b2"])
    nc = build_program(cfg, T, b1nz, b2nz)
    nc.finalize()
    core_ids = list(range(cfg.P))
    if sim:
        from concourse import bass_interp
        ms = bass_interp.MultiCoreSim(nc, cfg.P)
        for c in core_ids:
            for k, v in in_maps[c].items():
                ms.cores[c].tensor(k)[:] = v
        ms.simulate()
        outs = [np.array(ms.cores[c].tensor("out")) for c in core_ids]
        return np.concatenate(outs, axis=0), None
    res = run_bass_kernel_spmd(nc, in_maps, core_ids, trace=trace)
    outs = [np.asarray(res.results[c]["out"]) for c in core_ids]
    return np.concatenate(outs, axis=0), res


def kernel(x, edge_index, W1, b1, W2, b2):
    out, _ = run(FULL, dict(x=x, edge_index=edge_index, W1=W1, b1=b1,
                            W2=W2, b2=b2))
    return out



# revision 2
# speedup vs baseline: 1.0242x; 1.0242x over previous
"""GCN encoder (2-layer GCNConv) on 8 Trainium2 NeuronCores — v2.

Key design vs v1:
  - Layer 1 aggregates FIRST on the host-prescaled input xs = D^-1/2 x, which
    every core holds in full (full_io): no phase-A matmul and no AllGather on
    the critical path; per-edge gathers start immediately.
        agg_x = (A+I) xs          (gather 1KB rows + one-hot segment matmuls)
        h1pre = relu(dinv * (dinv*agg_x @ W1 + b1))   (dinv folded via relu)
        u2    = h1pre @ W2        (transform-first for layer 2)
  - Layer 2: AllGather(u2) chunked in two window-aligned pieces so the
    collective hides under the layer-1 gather stream; per-edge gathers of
    256B rows + one-hot segment matmuls, two-pass accumulation (chunk 0 into
    SBUF f32, chunk 1 added from PSUM).
  - All gather padding uses trailing -1 indices (skipped by the Q7 ucode
    before descriptor generation) and gather buffers are memset once so
    skipped slots never contain NaNs.
"""

import math
import sys

import numpy as np

sys.path.insert(0, "/opt/trn_rl_repo")

import ml_dtypes

BF16 = ml_dtypes.bfloat16
FP8 = ml_dtypes.float8_e4m3


class Cfg:
    def __init__(self, N=50000, E=800000, IN=512, HID=256, OUT=128, P=8):
        self.N, self.E, self.IN, self.HID, self.OUT, self.P = N, E, IN, HID, OUT, P
        self.NC = N // P                       # 6250 nodes per core
        self.WS = 128
        self.NW = math.ceil(self.NC / self.WS)  # 49 windows
        self.HALF = (N + 1) // 2                # layer-1 src halves (int16)
        # layer-2 chunk boundary: window-aligned split of the NC rows
        self.CW0 = (self.NW + 1) // 2           # 25 windows -> rows [0, 3200)
        self.C0 = self.CW0 * self.WS            # 3200
        self.C1 = self.NC - self.C0             # 3050


FULL = Cfg()


def _group_edges(cfg, src, dst, key_src, n_groups, windows, cores):
    """Sort edges by (core, window, group(src)); return per-core group counts
    and the sorted arrays."""
    NC, WS, NW = cfg.NC, cfg.WS, cfg.NW
    win = (dst % NC) // WS
    comp = (cores * NW + win) * n_groups + key_src
    order = np.argsort(comp, kind="stable")
    return src[order], dst[order], comp[order]


def _prepare(cfg, x, edge_index, W1, b1, W2, b2):
    N, P, NC, WS, NW = cfg.N, cfg.P, cfg.NC, cfg.WS, cfg.NW
    IN, HID, OUT = cfg.IN, cfg.HID, cfg.OUT
    src = np.asarray(edge_index[0], dtype=np.int64)
    dst = np.asarray(edge_index[1], dtype=np.int64)

    deg = np.bincount(dst, minlength=N).astype(np.float64) + 1.0
    dinv = (1.0 / np.sqrt(deg)).astype(np.float32)

    xs = np.clip(np.asarray(x, np.float32) * dinv[:, None],
                 -240.0, 240.0).astype(FP8)  # [N, IN] fp8

    cores = dst // NC

    def build_groups(key_src, n_groups, idx_of_src):
        s_s, d_s, c_s = _group_edges(cfg, src, dst, key_src, n_groups, None, cores)
        counts = np.bincount(c_s, minlength=P * NW * n_groups)
        counts = counts.reshape(P, NW, n_groups)
        T = np.ceil(counts.max(axis=0) / 128).astype(np.int64)   # [NW, n_groups]
        starts = np.zeros(P * NW * n_groups + 1, dtype=np.int64)
        np.cumsum(counts.reshape(-1), out=starts[1:])
        tiles_total = int(T.sum())
        slots_total = tiles_total * 128
        idxs, acols, cntsl = [], [], []
        for c in range(P):
            idx_arr = np.zeros(slots_total, dtype=np.int16)
            aco_arr = np.full(slots_total, -1, dtype=np.float32)
            off = 0
            for w in range(NW):
                for g in range(n_groups):
                    gi = (c * NW + w) * n_groups + g
                    n = counts[c, w, g]
                    sl = slice(starts[gi], starts[gi] + n)
                    idx_arr[off:off + n] = idx_of_src(s_s[sl], g)
                    aco_arr[off:off + n] = (d_s[sl] - c * NC - w * WS).astype(np.float32)
                    off += 128 * int(T[w, g])
            assert off == slots_total
            idxs.append(idx_arr)
            acols.append(aco_arr)
        return T, idxs, acols

    # layer 1: groups by src half (gather from xs halves)
    half = (src >= cfg.HALF).astype(np.int64)
    T1, idx1, aco1 = build_groups(
        half, 2, lambda s, g: (s - g * cfg.HALF).astype(np.int16))

    # layer 2: groups by within-core chunk (gather from U2a/U2b)
    srcl = src % NC
    chunk = (srcl >= cfg.C0).astype(np.int64)
    srccore = src // NC

    def idx2(s, g):
        sc = s // NC
        sl = s % NC
        base = cfg.C0 if g == 0 else cfg.C1
        return (sc * base + (sl - g * cfg.C0)).astype(np.int16)

    T2, idx2s, aco2 = build_groups(chunk, 2, idx2)

    def tile_idx(a):
        return np.ascontiguousarray(np.tile(a.reshape(-1, 16).T, (8, 1)))

    in_maps = []
    b1nz = bool(np.any(np.asarray(b1)))
    b2nz = bool(np.any(np.asarray(b2)))
    for c in range(P):
        dloc = np.concatenate(
            [dinv[c * NC:(c + 1) * NC], np.ones(NW * WS - NC, dtype=np.float32)])
        m = {
            "xs": xs,  # full prescaled input, every core
            "xw": np.ascontiguousarray(np.concatenate(
                [xs[c * NC:(c + 1) * NC],
                 np.zeros((NW * WS - NC, IN), dtype=FP8)])),
            "w1": np.ascontiguousarray(
                np.asarray(W1, np.float32).astype(BF16)
                .reshape(IN // 128, 128, HID).transpose(1, 0, 2)),
            "w2": np.ascontiguousarray(
                np.asarray(W2, np.float32).astype(BF16)
                .reshape(HID // 128, 128, OUT).transpose(1, 0, 2)),
            "dinvc": np.ascontiguousarray(dloc.reshape(NW, WS).T),
            "idx1": tile_idx(idx1[c]),
            "acol1": np.ascontiguousarray(
                aco1[c].reshape(-1, 128).T.astype(BF16)),
            "idx2": tile_idx(idx2s[c]),
            "acol2": np.ascontiguousarray(
                aco2[c].reshape(-1, 128).T.astype(BF16)),
            "ident": np.eye(128, dtype=BF16),
            "ident8": np.eye(128, dtype=FP8),
        }
        if b1nz:
            m["b1bc"] = np.ascontiguousarray(
                np.broadcast_to(np.asarray(b1, np.float32), (128, HID)))
        if b2nz:
            m["b2bc"] = np.ascontiguousarray(
                np.broadcast_to(np.asarray(b2, np.float32), (128, OUT)))
        in_maps.append(m)

    return in_maps, T1, T2, b1nz, b2nz


def build_program(cfg, T1, T2, b1nz, b2nz):
    import concourse.bass as bass
    import concourse.bacc as bacc
    import concourse.mybir as mybir
    from concourse import tile

    N, P, NC, WS, NW = cfg.N, cfg.P, cfg.NC, cfg.WS, cfg.NW
    IN, HID, OUT = cfg.IN, cfg.HID, cfg.OUT
    NCI, NCH = IN // 128, HID // 128
    t1_total = int(T1.sum())
    s1_total = t1_total * 128
    t2_total = int(T2.sum())
    s2_total = t2_total * 128
    f32, bf16, i16 = mybir.dt.float32, mybir.dt.bfloat16, mybir.dt.int16
    fp8 = mybir.dt.float8e4
    AF = mybir.ActivationFunctionType

    nc = bacc.Bacc("TRN2", target_bir_lowering=False, debug=False,
                   num_devices=cfg.P, num_swdge_queues=4)
    xs_p = nc.dram_tensor("xs", [N, IN], fp8, kind="ExternalInput")
    xw_p = nc.dram_tensor("xw", [NW * WS, IN], fp8, kind="ExternalInput")
    w1_p = nc.dram_tensor("w1", [128, NCI, HID], bf16, kind="ExternalInput")
    w2_p = nc.dram_tensor("w2", [128, NCH, OUT], bf16, kind="ExternalInput")
    dinv_p = nc.dram_tensor("dinvc", [WS, NW], f32, kind="ExternalInput")
    idx1_p = nc.dram_tensor("idx1", [128, s1_total // 16], i16, kind="ExternalInput")
    aco1_p = nc.dram_tensor("acol1", [128, t1_total], bf16, kind="ExternalInput")
    idx2_p = nc.dram_tensor("idx2", [128, s2_total // 16], i16, kind="ExternalInput")
    aco2_p = nc.dram_tensor("acol2", [128, t2_total], bf16, kind="ExternalInput")
    id_p = nc.dram_tensor("ident", [128, 128], bf16, kind="ExternalInput")
    id8_p = nc.dram_tensor("ident8", [128, 128], fp8, kind="ExternalInput")
    b1_p = (nc.dram_tensor("b1bc", [128, HID], f32, kind="ExternalInput")
            if b1nz else None)
    b2_p = (nc.dram_tensor("b2bc", [128, OUT], f32, kind="ExternalInput")
            if b2nz else None)
    out_p = nc.dram_tensor("out", [NC, OUT], f32, kind="ExternalOutput")

    u2da = nc.dram_tensor("u2da", [cfg.C0, OUT], bf16)
    u2db = nc.dram_tensor("u2db", [cfg.C1, OUT], bf16)
    U2a = nc.dram_tensor("U2a", [P * cfg.C0, OUT], bf16, addr_space="Shared")
    U2b = nc.dram_tensor("U2b", [P * cfg.C1, OUT], bf16, addr_space="Shared")
    rg = [list(range(P))]

    with tile.TileContext(nc) as tc:
        with (
            tc.tile_pool(name="res", bufs=1) as res,
            tc.tile_pool(name="gath", bufs=6) as gath,
            tc.tile_pool(name="work", bufs=4) as work,
            tc.tile_pool(name="spool", bufs=6) as spool,
            tc.tile_pool(name="psA", bufs=2, space="PSUM") as psA,
            tc.tile_pool(name="psB", bufs=2, space="PSUM") as psB,
        ):
            # ---- resident loads (small; gathers only need idx tensors) ----
            idx1s = res.tile([128, s1_total // 16], i16)
            nc.sync.dma_start(idx1s[:], idx1_p[:])
            idx2s = res.tile([128, s2_total // 16], i16)
            nc.sync.dma_start(idx2s[:], idx2_p[:])
            aco1s = res.tile([128, t1_total], bf16)
            nc.sync.dma_start(aco1s[:], aco1_p[:])
            aco2s = res.tile([128, t2_total], bf16)
            nc.sync.dma_start(aco2s[:], aco2_p[:])
            w1s = res.tile([128, NCI, HID], bf16)
            nc.sync.dma_start(w1s[:], w1_p[:])
            w2s = res.tile([128, NCH, OUT], bf16)
            nc.sync.dma_start(w2s[:], w2_p[:])
            dinvs = res.tile([WS, NW], f32)
            nc.sync.dma_start(dinvs[:], dinv_p[:])
            ident = res.tile([128, 128], bf16)
            nc.sync.dma_start(ident[:], id_p[:])
            ident8 = res.tile([128, 128], fp8)
            nc.sync.dma_start(ident8[:], id8_p[:])
            qrr = [0]
            iot4 = res.tile([128, 4, 128], bf16)
            nc.gpsimd.iota(iot4[:], pattern=[[0, 4], [1, 128]], base=0,
                           channel_multiplier=0,
                           allow_small_or_imprecise_dtypes=True)
            b1bc = None
            if b1nz:
                b1bc = res.tile([128, HID], f32)
                nc.sync.dma_start(b1bc[:], b1_p[:])
            b2bc = None
            if b2nz:
                b2bc = res.tile([128, OUT], f32)
                nc.sync.dma_start(b2bc[:], b2_p[:])

            u2res = res.tile([128, NW, OUT], bf16)
            acc2 = res.tile([128, NW, OUT], f32)   # layer-2 chunk-0 partials
            if NC % WS:
                nc.gpsimd.memset(u2res[:, NW - 1, :], 0.0)

            T1max = int(T1.max())
            T2max = int(T2.max())

            def nsz(j):
                return min(128, NC - j * WS)

            # ================= layer 1 =================
            slot_off = [0]
            tile_ix = [0]

            def agg_matmuls(acols, g, t_cnt, pa, n, done0, nmm, tag,
                            sdt=bf16):
                """Emit batched S-gen + one matmul per tile; returns done."""
                done = done0
                t = 0
                while t < t_cnt:
                    bsz = min(4, t_cnt - t)
                    S4 = spool.tile([128, 4, 128], sdt, tag=tag, name="S4" + tag)
                    nc.vector.tensor_tensor(
                        S4[:, :bsz, :], iot4[:, :bsz, :],
                        acols[:, tile_ix[0] + t:tile_ix[0] + t + bsz]
                        .broadcast_to((128, bsz, 128)),
                        op=mybir.AluOpType.is_equal)
                    for j in range(bsz):
                        done += 1
                        nc.tensor.matmul(pa[:n, :], S4[:, j, :n], g[:, t + j, :],
                                         start=False, stop=(done == nmm))
                    t += bsz
                tile_ix[0] += t_cnt
                return done

            for w in range(NW):
                n = nsz(w)
                wsl = slice(w * WS, w * WS + n)
                # self-loop rows (contiguous load of local xs window)
                xw = work.tile([128, IN], fp8, tag="xw")
                nc.sync.dma_start(xw[:], xw_p[w * WS:(w + 1) * WS, :])
                pa = psA.tile([128, IN], f32, tag="pa")
                nc.tensor.matmul(pa[:n, :], ident8[:, :n], xw[:, :],
                                 start=True, stop=False)
                nmm = int(T1[w, 0] + T1[w, 1])
                done = 0
                for h in range(2):
                    t_wh = int(T1[w, h])
                    if t_wh == 0:
                        continue
                    g = gath.tile([128, T1max, IN], fp8, tag="g1")
                    base = 0 if h == 0 else cfg.HALF
                    nc.gpsimd.dma_gather(
                        g[:, :t_wh, :], xs_p[base:base + min(cfg.HALF, N - base), :],
                        idx1s[:, slot_off[0] // 16:(slot_off[0] + 128 * t_wh) // 16],
                        num_idxs=128 * t_wh, num_idxs_reg=128 * t_wh,
                        elem_size=IN, single_packet=False,
                        queue_num=qrr[0] % 4)
                    qrr[0] += 1
                    slot_off[0] += 128 * t_wh
                    done = agg_matmuls(aco1s, g, t_wh, pa, n, done, nmm, "S1",
                                       sdt=fp8)
                # v = dinv * agg_x  (bf16)
                v = work.tile([128, IN], bf16, tag="v")
                nc.scalar.activation(v[:n, :], pa[:n, :], AF.Copy,
                                     scale=dinvs[:n, w:w + 1])
                # transpose v -> vT chunks, then t1 = vT.T @ W1
                pt1 = psB.tile([128, HID], f32, tag="pt")
                for ci in range(NCI):
                    ptr = psA.tile([128, 128], bf16, tag="tr")
                    nc.tensor.transpose(ptr[:, :n],
                                        v[:n, ci * 128:(ci + 1) * 128],
                                        ident[:n, :n])
                    vT = work.tile([128, 128], bf16, tag="vT")
                    nc.scalar.activation(vT[:, :n], ptr[:, :n], AF.Copy)
                    nc.tensor.matmul(pt1[:n, :], vT[:, :n], w1s[:, ci, :],
                                     start=(ci == 0), stop=(ci == NCI - 1))
                # h1pre = relu(dinv * t1 (+ dinv*b1))
                hp = work.tile([128, HID], bf16, tag="hp")
                if b1nz:
                    tmp = work.tile([128, HID], f32, tag="tmp")
                    nc.vector.tensor_tensor(tmp[:n, :], pt1[:n, :], b1bc[:n, :],
                                            op=mybir.AluOpType.add)
                    nc.scalar.activation(hp[:n, :], tmp[:n, :], AF.Relu,
                                         scale=dinvs[:n, w:w + 1])
                else:
                    nc.scalar.activation(hp[:n, :], pt1[:n, :], AF.Relu,
                                         scale=dinvs[:n, w:w + 1])
                # u2 = h1pre @ W2  (transpose h1pre chunks first)
                pt2f = psB.tile([128, HID], f32, tag="pt", name="pt2f")
                pt2 = pt2f[:, :OUT]
                for ch in range(NCH):
                    ptr = psA.tile([128, 128], bf16, tag="tr")
                    nc.tensor.transpose(ptr[:, :n],
                                        hp[:n, ch * 128:(ch + 1) * 128],
                                        ident[:n, :n])
                    hT = work.tile([128, 128], bf16, tag="hT")
                    nc.scalar.activation(hT[:, :n], ptr[:, :n], AF.Copy)
                    nc.tensor.matmul(pt2[:n, :], hT[:, :n], w2s[:, ch, :],
                                     start=(ch == 0), stop=(ch == NCH - 1))
                nc.scalar.activation(u2res[:n, w, :], pt2[:n, :], AF.Copy)
                if w < cfg.CW0:
                    nc.sync.dma_start(u2da[w * WS:w * WS + n, :],
                                      u2res[:n, w, :])
                else:
                    ww = w - cfg.CW0
                    nc.sync.dma_start(u2db[ww * WS:ww * WS + n, :],
                                      u2res[:n, w, :])
                if w == cfg.CW0 - 1:
                    nc.gpsimd.collective_compute(
                        "AllGather", mybir.AluOpType.bypass, replica_groups=rg,
                        ins=[u2da[:]], outs=[U2a[:]])
            nc.gpsimd.collective_compute(
                "AllGather", mybir.AluOpType.bypass, replica_groups=rg,
                ins=[u2db[:]], outs=[U2b[:]])

            # ================= layer 2 (two-pass over chunks) =================
            # pass structure: for chunk 0: all windows (selfloop + c0 tiles)
            # -> acc2; for chunk 1: all windows (c1 tiles + acc2) -> out.
            # idx/acol layout is grouped (w, chunk) contiguous; compute group
            # offsets host-side here.
            off_wc = np.zeros((NW, 2), dtype=np.int64)
            tix_wc = np.zeros((NW, 2), dtype=np.int64)
            o = 0
            tix = 0
            for w in range(NW):
                for g in range(2):
                    off_wc[w, g] = o
                    tix_wc[w, g] = tix
                    o += 128 * int(T2[w, g])
                    tix += int(T2[w, g])

            for cpass in range(2):
                for w in range(NW):
                    n = nsz(w)
                    t_wc = int(T2[w, cpass])
                    paf = psB.tile([128, HID], f32, tag="pt", name="paf")
                    pa = paf[:, :OUT]
                    first = True
                    if cpass == 0:
                        nc.tensor.matmul(pa[:n, :], ident[:, :n],
                                         u2res[:, w, :], start=True,
                                         stop=(t_wc == 0))
                        first = False
                    done = 0
                    if t_wc:
                        g = gath.tile([128, T2max, OUT], bf16, tag="g2")
                        U2c = U2a if cpass == 0 else U2b
                        so = int(off_wc[w, cpass])
                        nc.gpsimd.dma_gather(
                            g[:, :t_wc, :], U2c[:, :],
                            idx2s[:, so // 16:(so + 128 * t_wc) // 16],
                            num_idxs=128 * t_wc, num_idxs_reg=128 * t_wc,
                            elem_size=OUT, single_packet=False,
                            queue_num=qrr[0] % 4)
                        qrr[0] += 1
                        tile_ix[0] = int(tix_wc[w, cpass])
                        if first:
                            # no self-loop matmul opened the psum group: use
                            # the first S matmul as start
                            S0 = spool.tile([128, 4, 128], bf16, tag="S2",
                                            name="S4S2f")
                            nc.vector.tensor_tensor(
                                S0[:, 0:1, :], iot4[:, 0:1, :],
                                aco2s[:, tile_ix[0]:tile_ix[0] + 1]
                                .broadcast_to((128, 1, 128)),
                                op=mybir.AluOpType.is_equal)
                            done += 1
                            nc.tensor.matmul(pa[:n, :], S0[:, 0, :n], g[:, 0, :],
                                             start=True, stop=(done == t_wc))
                            tile_ix[0] += 1
                            done = agg_matmuls(aco2s, g[:, 1:, :], t_wc - 1,
                                               pa, n, done, t_wc, "S2")
                        else:
                            done = agg_matmuls(aco2s, g, t_wc, pa, n, done,
                                               t_wc, "S2")
                    if cpass == 0:
                        nc.scalar.activation(acc2[:n, w, :], pa[:n, :], AF.Copy)
                    else:
                        zt = work.tile([128, OUT], f32, tag="zt")
                        if t_wc:
                            nc.vector.tensor_tensor(
                                zt[:n, :], pa[:n, :], acc2[:n, w, :],
                                op=mybir.AluOpType.add)
                            zsrc = zt
                        else:
                            zsrc = acc2[:, w, :]
                        fin = work.tile([128, OUT], f32, tag="fin")
                        if b2nz:
                            v2 = work.tile([128, OUT], f32, tag="v2")
                            nc.scalar.activation(v2[:n, :],
                                                 zsrc[:n, :] if t_wc else acc2[:n, w, :],
                                                 AF.Copy,
                                                 scale=dinvs[:n, w:w + 1])
                            nc.vector.tensor_tensor(fin[:n, :], v2[:n, :],
                                                    b2bc[:n, :],
                                                    op=mybir.AluOpType.add)
                        else:
                            nc.scalar.activation(fin[:n, :],
                                                 zsrc[:n, :] if t_wc else acc2[:n, w, :],
                                                 AF.Copy,
                                                 scale=dinvs[:n, w:w + 1])
                        nc.sync.dma_start(out_p[w * WS:w * WS + n, :],
                                          fin[:n, :])

    return nc


def run(cfg, inputs, sim=False, trace=False):
    from concourse.bass_utils import run_bass_kernel_spmd

    in_maps, T1, T2, b1nz, b2nz = _prepare(
        cfg, inputs["x"], inputs["edge_index"], inputs["W1"], inputs["b1"],
        inputs["W2"], inputs["b2"])
    nc = build_program(cfg, T1, T2, b1nz, b2nz)
    nc.finalize()
    core_ids = list(range(cfg.P))
    if sim:
        from concourse import bass_interp
        ms = bass_interp.MultiCoreSim(nc, cfg.P)
        for c in core_ids:
            for k, v in in_maps[c].items():
                ms.cores[c].tensor(k)[:] = v
        ms.simulate()
        outs = [np.array(ms.cores[c].tensor("out")) for c in core_ids]
        return np.concatenate(outs, axis=0), None
    res = run_bass_kernel_spmd(nc, in_maps, core_ids, trace=trace)
    outs = [np.asarray(res.results[c]["out"]) for c in core_ids]
    return np.concatenate(outs, axis=0), res


def kernel(x, edge_index, W1, b1, W2, b2):
    out, _ = run(FULL, dict(x=x, edge_index=edge_index, W1=W1, b1=b1,
                            W2=W2, b2=b2))
    return out


# revision 3
# speedup vs baseline: 1.0486x; 1.0238x over previous
"""GCN encoder (2-layer GCNConv) on 8 Trainium2 NeuronCores.

Nodes are row-partitioned across the 8 cores (6250 rows each) and edges are
partitioned by destination; the per-edge segment-sum runs as one-hot
selection matmuls on the tensor engine.  The dominant cost is GPSIMD (Q7)
DMA-descriptor generation for the per-edge row gathers, so the design
minimizes and parallelizes exactly that:

  - Layer 1 aggregates FIRST on the host-prescaled input xs = D^-1/2 x
    (clipped to fp8 e4m3), which every core holds in full: no phase-A matmul
    and no AllGather on the critical path; per-edge gathers start at t=0.
        agg_x = (A+I) xs          (fp8 512B-row gathers + one-hot matmuls)
        h1pre = relu(dinv * (dinv*agg_x @ W1 + b1))   (dinv folded via relu)
        u2    = h1pre @ W2        (transform-first for layer 2)
  - dma_gather calls alternate between two SWDGE queues (strict rotation so
    Tile's DMASW semaphore-lane/queue binding stays consistent): descriptor
    generation for queue 0 runs on Q7 cores 0-1 and queue 1 on cores 2-3,
    overlapping ~2x.  Two queues (not four) leave the collectives a larger
    SDMA share.
  - Layer 2: AllGather(u2) is split into two window-aligned chunks whose
    triggers are interleaved into the layer-1 stream so the collectives hide
    under the gather work; 256B-row gathers + one-hot matmuls accumulate in
    two passes (chunk 0 -> SBUF f32 accumulator, chunk 1 added from PSUM).
  - S selection matrices are generated on the DVE four 128x128 tiles per
    is_equal op (iota vs broadcast dst-offset columns); fp8 output for the
    layer-1 matmuls (0/1 is exact in fp8).
"""

import math
import sys

import numpy as np

sys.path.insert(0, "/opt/trn_rl_repo")

import ml_dtypes

BF16 = ml_dtypes.bfloat16
FP8 = ml_dtypes.float8_e4m3


class Cfg:
    def __init__(self, N=50000, E=800000, IN=512, HID=256, OUT=128, P=8):
        self.N, self.E, self.IN, self.HID, self.OUT, self.P = N, E, IN, HID, OUT, P
        self.NC = N // P                       # 6250 nodes per core
        self.WS = 128
        self.NW = math.ceil(self.NC / self.WS)  # 49 windows
        self.HALF = (N + 1) // 2                # layer-1 src halves (int16)
        # layer-2 chunk boundary: window-aligned split of the NC rows
        self.CW0 = (self.NW + 1) // 2           # 25 windows -> rows [0, 3200)
        self.C0 = self.CW0 * self.WS            # 3200
        self.C1 = self.NC - self.C0             # 3050


FULL = Cfg()


def _group_edges(cfg, src, dst, key_src, n_groups, windows, cores):
    """Sort edges by (core, window, group(src)); return per-core group counts
    and the sorted arrays."""
    NC, WS, NW = cfg.NC, cfg.WS, cfg.NW
    win = (dst % NC) // WS
    comp = (cores * NW + win) * n_groups + key_src
    order = np.argsort(comp, kind="stable")
    return src[order], dst[order], comp[order]


def _prepare(cfg, x, edge_index, W1, b1, W2, b2):
    N, P, NC, WS, NW = cfg.N, cfg.P, cfg.NC, cfg.WS, cfg.NW
    IN, HID, OUT = cfg.IN, cfg.HID, cfg.OUT
    src = np.asarray(edge_index[0], dtype=np.int64)
    dst = np.asarray(edge_index[1], dtype=np.int64)

    deg = np.bincount(dst, minlength=N).astype(np.float64) + 1.0
    dinv = (1.0 / np.sqrt(deg)).astype(np.float32)

    xs = np.clip(np.asarray(x, np.float32) * dinv[:, None],
                 -240.0, 240.0).astype(FP8)  # [N, IN] fp8

    cores = dst // NC

    def build_groups(key_src, n_groups, idx_of_src):
        s_s, d_s, c_s = _group_edges(cfg, src, dst, key_src, n_groups, None, cores)
        counts = np.bincount(c_s, minlength=P * NW * n_groups)
        counts = counts.reshape(P, NW, n_groups)
        T = np.ceil(counts.max(axis=0) / 128).astype(np.int64)   # [NW, n_groups]
        starts = np.zeros(P * NW * n_groups + 1, dtype=np.int64)
        np.cumsum(counts.reshape(-1), out=starts[1:])
        tiles_total = int(T.sum())
        slots_total = tiles_total * 128
        idxs, acols, cntsl = [], [], []
        for c in range(P):
            idx_arr = np.zeros(slots_total, dtype=np.int16)
            aco_arr = np.full(slots_total, -1, dtype=np.float32)
            off = 0
            for w in range(NW):
                for g in range(n_groups):
                    gi = (c * NW + w) * n_groups + g
                    n = counts[c, w, g]
                    sl = slice(starts[gi], starts[gi] + n)
                    idx_arr[off:off + n] = idx_of_src(s_s[sl], g)
                    aco_arr[off:off + n] = (d_s[sl] - c * NC - w * WS).astype(np.float32)
                    off += 128 * int(T[w, g])
            assert off == slots_total
            idxs.append(idx_arr)
            acols.append(aco_arr)
        return T, idxs, acols

    # layer 1: groups by src half (gather from xs halves)
    half = (src >= cfg.HALF).astype(np.int64)
    T1, idx1, aco1 = build_groups(
        half, 2, lambda s, g: (s - g * cfg.HALF).astype(np.int16))

    # layer 2: groups by within-core chunk (gather from U2a/U2b)
    srcl = src % NC
    chunk = (srcl >= cfg.C0).astype(np.int64)
    srccore = src // NC

    def idx2(s, g):
        sc = s // NC
        sl = s % NC
        base = cfg.C0 if g == 0 else cfg.C1
        return (sc * base + (sl - g * cfg.C0)).astype(np.int16)

    T2, idx2s, aco2 = build_groups(chunk, 2, idx2)

    def tile_idx(a):
        return np.ascontiguousarray(np.tile(a.reshape(-1, 16).T, (8, 1)))

    in_maps = []
    b1nz = bool(np.any(np.asarray(b1)))
    b2nz = bool(np.any(np.asarray(b2)))
    for c in range(P):
        dloc = np.concatenate(
            [dinv[c * NC:(c + 1) * NC], np.ones(NW * WS - NC, dtype=np.float32)])
        m = {
            "xs": xs,  # full prescaled input, every core
            "xw": np.ascontiguousarray(np.concatenate(
                [xs[c * NC:(c + 1) * NC],
                 np.zeros((NW * WS - NC, IN), dtype=FP8)])),
            "w1": np.ascontiguousarray(
                np.asarray(W1, np.float32).astype(BF16)
                .reshape(IN // 128, 128, HID).transpose(1, 0, 2)),
            "w2": np.ascontiguousarray(
                np.asarray(W2, np.float32).astype(BF16)
                .reshape(HID // 128, 128, OUT).transpose(1, 0, 2)),
            "dinvc": np.ascontiguousarray(dloc.reshape(NW, WS).T),
            "idx1": tile_idx(idx1[c]),
            "acol1": np.ascontiguousarray(
                aco1[c].reshape(-1, 128).T.astype(BF16)),
            "idx2": tile_idx(idx2s[c]),
            "acol2": np.ascontiguousarray(
                aco2[c].reshape(-1, 128).T.astype(BF16)),
            "ident": np.eye(128, dtype=BF16),
            "ident8": np.eye(128, dtype=FP8),
        }
        if b1nz:
            m["b1bc"] = np.ascontiguousarray(
                np.broadcast_to(np.asarray(b1, np.float32), (128, HID)))
        if b2nz:
            m["b2bc"] = np.ascontiguousarray(
                np.broadcast_to(np.asarray(b2, np.float32), (128, OUT)))
        in_maps.append(m)

    return in_maps, T1, T2, b1nz, b2nz


def build_program(cfg, T1, T2, b1nz, b2nz):
    import concourse.bass as bass
    import concourse.bacc as bacc
    import concourse.mybir as mybir
    from concourse import tile

    N, P, NC, WS, NW = cfg.N, cfg.P, cfg.NC, cfg.WS, cfg.NW
    IN, HID, OUT = cfg.IN, cfg.HID, cfg.OUT
    NCI, NCH = IN // 128, HID // 128
    t1_total = int(T1.sum())
    s1_total = t1_total * 128
    t2_total = int(T2.sum())
    s2_total = t2_total * 128
    f32, bf16, i16 = mybir.dt.float32, mybir.dt.bfloat16, mybir.dt.int16
    fp8 = mybir.dt.float8e4
    AF = mybir.ActivationFunctionType

    nc = bacc.Bacc("TRN2", target_bir_lowering=False, debug=False,
                   num_devices=cfg.P, num_swdge_queues=4)
    xs_p = nc.dram_tensor("xs", [N, IN], fp8, kind="ExternalInput")
    xw_p = nc.dram_tensor("xw", [NW * WS, IN], fp8, kind="ExternalInput")
    w1_p = nc.dram_tensor("w1", [128, NCI, HID], bf16, kind="ExternalInput")
    w2_p = nc.dram_tensor("w2", [128, NCH, OUT], bf16, kind="ExternalInput")
    dinv_p = nc.dram_tensor("dinvc", [WS, NW], f32, kind="ExternalInput")
    idx1_p = nc.dram_tensor("idx1", [128, s1_total // 16], i16, kind="ExternalInput")
    aco1_p = nc.dram_tensor("acol1", [128, t1_total], bf16, kind="ExternalInput")
    idx2_p = nc.dram_tensor("idx2", [128, s2_total // 16], i16, kind="ExternalInput")
    aco2_p = nc.dram_tensor("acol2", [128, t2_total], bf16, kind="ExternalInput")
    id_p = nc.dram_tensor("ident", [128, 128], bf16, kind="ExternalInput")
    id8_p = nc.dram_tensor("ident8", [128, 128], fp8, kind="ExternalInput")
    b1_p = (nc.dram_tensor("b1bc", [128, HID], f32, kind="ExternalInput")
            if b1nz else None)
    b2_p = (nc.dram_tensor("b2bc", [128, OUT], f32, kind="ExternalInput")
            if b2nz else None)
    out_p = nc.dram_tensor("out", [NC, OUT], f32, kind="ExternalOutput")

    u2da = nc.dram_tensor("u2da", [cfg.C0, OUT], bf16)
    u2db = nc.dram_tensor("u2db", [cfg.C1, OUT], bf16)
    U2a = nc.dram_tensor("U2a", [P * cfg.C0, OUT], bf16, addr_space="Shared")
    U2b = nc.dram_tensor("U2b", [P * cfg.C1, OUT], bf16, addr_space="Shared")
    rg = [list(range(P))]

    with tile.TileContext(nc) as tc:
        with (
            tc.tile_pool(name="res", bufs=1) as res,
            tc.tile_pool(name="gath", bufs=6) as gath,
            tc.tile_pool(name="work", bufs=4) as work,
            tc.tile_pool(name="spool", bufs=6) as spool,
            tc.tile_pool(name="psA", bufs=2, space="PSUM") as psA,
            tc.tile_pool(name="psB", bufs=2, space="PSUM") as psB,
        ):
            # ---- resident loads (small; gathers only need idx tensors) ----
            idx1s = res.tile([128, s1_total // 16], i16)
            nc.sync.dma_start(idx1s[:], idx1_p[:])
            idx2s = res.tile([128, s2_total // 16], i16)
            nc.sync.dma_start(idx2s[:], idx2_p[:])
            aco1s = res.tile([128, t1_total], bf16)
            nc.sync.dma_start(aco1s[:], aco1_p[:])
            aco2s = res.tile([128, t2_total], bf16)
            nc.sync.dma_start(aco2s[:], aco2_p[:])
            w1s = res.tile([128, NCI, HID], bf16)
            nc.sync.dma_start(w1s[:], w1_p[:])
            w2s = res.tile([128, NCH, OUT], bf16)
            nc.sync.dma_start(w2s[:], w2_p[:])
            dinvs = res.tile([WS, NW], f32)
            nc.sync.dma_start(dinvs[:], dinv_p[:])
            ident = res.tile([128, 128], bf16)
            nc.sync.dma_start(ident[:], id_p[:])
            ident8 = res.tile([128, 128], fp8)
            nc.sync.dma_start(ident8[:], id8_p[:])
            qrr = [0]
            iot4 = res.tile([128, 4, 128], bf16)
            nc.gpsimd.iota(iot4[:], pattern=[[0, 4], [1, 128]], base=0,
                           channel_multiplier=0,
                           allow_small_or_imprecise_dtypes=True)
            b1bc = None
            if b1nz:
                b1bc = res.tile([128, HID], f32)
                nc.sync.dma_start(b1bc[:], b1_p[:])
            b2bc = None
            if b2nz:
                b2bc = res.tile([128, OUT], f32)
                nc.sync.dma_start(b2bc[:], b2_p[:])

            u2res = res.tile([128, NW, OUT], bf16)
            acc2 = res.tile([128, NW, OUT], f32)   # layer-2 chunk-0 partials
            if NC % WS:
                nc.gpsimd.memset(u2res[:, NW - 1, :], 0.0)

            T1max = int(T1.max())
            T2max = int(T2.max())

            def nsz(j):
                return min(128, NC - j * WS)

            # ================= layer 1 =================
            slot_off = [0]
            tile_ix = [0]

            def agg_matmuls(acols, g, t_cnt, pa, n, done0, nmm, tag,
                            sdt=bf16):
                """Emit batched S-gen + one matmul per tile; returns done."""
                done = done0
                t = 0
                while t < t_cnt:
                    bsz = min(4, t_cnt - t)
                    S4 = spool.tile([128, 4, 128], sdt, tag=tag, name="S4" + tag)
                    nc.vector.tensor_tensor(
                        S4[:, :bsz, :], iot4[:, :bsz, :],
                        acols[:, tile_ix[0] + t:tile_ix[0] + t + bsz]
                        .broadcast_to((128, bsz, 128)),
                        op=mybir.AluOpType.is_equal)
                    for j in range(bsz):
                        done += 1
                        nc.tensor.matmul(pa[:n, :], S4[:, j, :n], g[:, t + j, :],
                                         start=False, stop=(done == nmm))
                    t += bsz
                tile_ix[0] += t_cnt
                return done

            for w in range(NW):
                n = nsz(w)
                wsl = slice(w * WS, w * WS + n)
                # self-loop rows (contiguous load of local xs window)
                xw = work.tile([128, IN], fp8, tag="xw")
                nc.sync.dma_start(xw[:], xw_p[w * WS:(w + 1) * WS, :])
                pa = psA.tile([128, IN], f32, tag="pa")
                nc.tensor.matmul(pa[:n, :], ident8[:, :n], xw[:, :],
                                 start=True, stop=False)
                nmm = int(T1[w, 0] + T1[w, 1])
                done = 0
                for h in range(2):
                    t_wh = int(T1[w, h])
                    if t_wh == 0:
                        continue
                    g = gath.tile([128, T1max, IN], fp8, tag="g1")
                    base = 0 if h == 0 else cfg.HALF
                    nc.gpsimd.dma_gather(
                        g[:, :t_wh, :], xs_p[base:base + min(cfg.HALF, N - base), :],
                        idx1s[:, slot_off[0] // 16:(slot_off[0] + 128 * t_wh) // 16],
                        num_idxs=128 * t_wh, num_idxs_reg=128 * t_wh,
                        elem_size=IN, single_packet=False,
                        queue_num=qrr[0] % 2)
                    qrr[0] += 1
                    slot_off[0] += 128 * t_wh
                    done = agg_matmuls(aco1s, g, t_wh, pa, n, done, nmm, "S1",
                                       sdt=fp8)
                # v = dinv * agg_x  (bf16)
                v = work.tile([128, IN], bf16, tag="v")
                nc.scalar.activation(v[:n, :], pa[:n, :], AF.Copy,
                                     scale=dinvs[:n, w:w + 1])
                # transpose v -> vT chunks, then t1 = vT.T @ W1
                pt1 = psB.tile([128, HID], f32, tag="pt")
                for ci in range(NCI):
                    ptr = psA.tile([128, 128], bf16, tag="tr")
                    nc.tensor.transpose(ptr[:, :n],
                                        v[:n, ci * 128:(ci + 1) * 128],
                                        ident[:n, :n])
                    vT = work.tile([128, 128], bf16, tag="vT")
                    nc.scalar.activation(vT[:, :n], ptr[:, :n], AF.Copy)
                    nc.tensor.matmul(pt1[:n, :], vT[:, :n], w1s[:, ci, :],
                                     start=(ci == 0), stop=(ci == NCI - 1))
                # h1pre = relu(dinv * t1 (+ dinv*b1))
                hp = work.tile([128, HID], bf16, tag="hp")
                if b1nz:
                    tmp = work.tile([128, HID], f32, tag="tmp")
                    nc.vector.tensor_tensor(tmp[:n, :], pt1[:n, :], b1bc[:n, :],
                                            op=mybir.AluOpType.add)
                    nc.scalar.activation(hp[:n, :], tmp[:n, :], AF.Relu,
                                         scale=dinvs[:n, w:w + 1])
                else:
                    nc.scalar.activation(hp[:n, :], pt1[:n, :], AF.Relu,
                                         scale=dinvs[:n, w:w + 1])
                # u2 = h1pre @ W2  (transpose h1pre chunks first)
                pt2f = psB.tile([128, HID], f32, tag="pt", name="pt2f")
                pt2 = pt2f[:, :OUT]
                for ch in range(NCH):
                    ptr = psA.tile([128, 128], bf16, tag="tr")
                    nc.tensor.transpose(ptr[:, :n],
                                        hp[:n, ch * 128:(ch + 1) * 128],
                                        ident[:n, :n])
                    hT = work.tile([128, 128], bf16, tag="hT")
                    nc.scalar.activation(hT[:, :n], ptr[:, :n], AF.Copy)
                    nc.tensor.matmul(pt2[:n, :], hT[:, :n], w2s[:, ch, :],
                                     start=(ch == 0), stop=(ch == NCH - 1))
                nc.scalar.activation(u2res[:n, w, :], pt2[:n, :], AF.Copy)
                if w < cfg.CW0:
                    nc.sync.dma_start(u2da[w * WS:w * WS + n, :],
                                      u2res[:n, w, :])
                else:
                    ww = w - cfg.CW0
                    nc.sync.dma_start(u2db[ww * WS:ww * WS + n, :],
                                      u2res[:n, w, :])
                if w == cfg.CW0 - 1:
                    nc.gpsimd.collective_compute(
                        "AllGather", mybir.AluOpType.bypass, replica_groups=rg,
                        ins=[u2da[:]], outs=[U2a[:]])
            nc.gpsimd.collective_compute(
                "AllGather", mybir.AluOpType.bypass, replica_groups=rg,
                ins=[u2db[:]], outs=[U2b[:]])

            # ================= layer 2 (two-pass over chunks) =================
            # pass structure: for chunk 0: all windows (selfloop + c0 tiles)
            # -> acc2; for chunk 1: all windows (c1 tiles + acc2) -> out.
            # idx/acol layout is grouped (w, chunk) contiguous; compute group
            # offsets host-side here.
            off_wc = np.zeros((NW, 2), dtype=np.int64)
            tix_wc = np.zeros((NW, 2), dtype=np.int64)
            o = 0
            tix = 0
            for w in range(NW):
                for g in range(2):
                    off_wc[w, g] = o
                    tix_wc[w, g] = tix
                    o += 128 * int(T2[w, g])
                    tix += int(T2[w, g])

            for cpass in range(2):
                for w in range(NW):
                    n = nsz(w)
                    t_wc = int(T2[w, cpass])
                    paf = psB.tile([128, HID], f32, tag="pt", name="paf")
                    pa = paf[:, :OUT]
                    first = True
                    if cpass == 0:
                        nc.tensor.matmul(pa[:n, :], ident[:, :n],
                                         u2res[:, w, :], start=True,
                                         stop=(t_wc == 0))
                        first = False
                    done = 0
                    if t_wc:
                        g = gath.tile([128, T2max, OUT], bf16, tag="g2")
                        U2c = U2a if cpass == 0 else U2b
                        so = int(off_wc[w, cpass])
                        nc.gpsimd.dma_gather(
                            g[:, :t_wc, :], U2c[:, :],
                            idx2s[:, so // 16:(so + 128 * t_wc) // 16],
                            num_idxs=128 * t_wc, num_idxs_reg=128 * t_wc,
                            elem_size=OUT, single_packet=False,
                            queue_num=qrr[0] % 2)
                        qrr[0] += 1
                        tile_ix[0] = int(tix_wc[w, cpass])
                        if first:
                            # no self-loop matmul opened the psum group: use
                            # the first S matmul as start
                            S0 = spool.tile([128, 4, 128], bf16, tag="S2",
                                            name="S4S2f")
                            nc.vector.tensor_tensor(
                                S0[:, 0:1, :], iot4[:, 0:1, :],
                                aco2s[:, tile_ix[0]:tile_ix[0] + 1]
                                .broadcast_to((128, 1, 128)),
                                op=mybir.AluOpType.is_equal)
                            done += 1
                            nc.tensor.matmul(pa[:n, :], S0[:, 0, :n], g[:, 0, :],
                                             start=True, stop=(done == t_wc))
                            tile_ix[0] += 1
                            done = agg_matmuls(aco2s, g[:, 1:, :], t_wc - 1,
                                               pa, n, done, t_wc, "S2")
                        else:
                            done = agg_matmuls(aco2s, g, t_wc, pa, n, done,
                                               t_wc, "S2")
                    if cpass == 0:
                        nc.scalar.activation(acc2[:n, w, :], pa[:n, :], AF.Copy)
                    else:
                        zt = work.tile([128, OUT], f32, tag="zt")
                        if t_wc:
                            nc.vector.tensor_tensor(
                                zt[:n, :], pa[:n, :], acc2[:n, w, :],
                                op=mybir.AluOpType.add)
                            zsrc = zt
                        else:
                            zsrc = acc2[:, w, :]
                        fin = work.tile([128, OUT], f32, tag="fin")
                        if b2nz:
                            v2 = work.tile([128, OUT], f32, tag="v2")
                            nc.scalar.activation(v2[:n, :],
                                                 zsrc[:n, :] if t_wc else acc2[:n, w, :],
                                                 AF.Copy,
                                                 scale=dinvs[:n, w:w + 1])
                            nc.vector.tensor_tensor(fin[:n, :], v2[:n, :],
                                                    b2bc[:n, :],
                                                    op=mybir.AluOpType.add)
                        else:
                            nc.scalar.activation(fin[:n, :],
                                                 zsrc[:n, :] if t_wc else acc2[:n, w, :],
                                                 AF.Copy,
                                                 scale=dinvs[:n, w:w + 1])
                        nc.sync.dma_start(out_p[w * WS:w * WS + n, :],
                                          fin[:n, :])

    return nc


def run(cfg, inputs, sim=False, trace=False):
    from concourse.bass_utils import run_bass_kernel_spmd

    in_maps, T1, T2, b1nz, b2nz = _prepare(
        cfg, inputs["x"], inputs["edge_index"], inputs["W1"], inputs["b1"],
        inputs["W2"], inputs["b2"])
    nc = build_program(cfg, T1, T2, b1nz, b2nz)
    nc.finalize()
    core_ids = list(range(cfg.P))
    if sim:
        from concourse import bass_interp
        ms = bass_interp.MultiCoreSim(nc, cfg.P)
        for c in core_ids:
            for k, v in in_maps[c].items():
                ms.cores[c].tensor(k)[:] = v
        ms.simulate()
        outs = [np.array(ms.cores[c].tensor("out")) for c in core_ids]
        return np.concatenate(outs, axis=0), None
    res = run_bass_kernel_spmd(nc, in_maps, core_ids, trace=trace)
    outs = [np.asarray(res.results[c]["out"]) for c in core_ids]
    return np.concatenate(outs, axis=0), res


def kernel(x, edge_index, W1, b1, W2, b2):
    out, _ = run(FULL, dict(x=x, edge_index=edge_index, W1=W1, b1=b1,
                            W2=W2, b2=b2))
    return out
